# revision 1
# baseline (speedup 1.0000x reference)
"""AnalyticGaussianVelocity (soft-kNN flow velocity) on 8 trn2 NeuronCores.

Math (reference):
    a = t, b = 1-t
    logit[b,n] = -1/(2 b^2) * ||x_b - a * d_n||^2
    prob = softmax(logit, axis=n) * (1 + a/b)
    v = (-1/b) x + prob @ dataset

Dropping per-row constants, softmax(logit) == softmax(u * P) with
    u = a/b^2  (>0),  P[b,n] = x_b . d_n - (a/2) ||d_n||^2

Kernel strategy (dataset sharded over N across 8 cores, flash-style
online softmax per core, AllReduce merge):
  MM1: P = x^T . dataT as a 3-pass hi/lo bf16 split ("bsplit" default,
       1 cyc/row and interleave-safe; "fp32" = plain fp32 at 4 cyc/row;
       "split" = 3-pass split-float32r - fastest per-op but f32r
       accumulation groups get corrupted when fp32 transpose-mode PE ops
       interleave with them, do not enable without revalidating) +
       a K=6 matmul folding in the -(a/2)||d||^2 term from 3-way bf16
       splits of w and of the dataset norms (norms via fp32 ones-matmul
       on squared transposed chunks).
  softmax: DVE row-max -> ACT exp(scale=u, bias=-u*m) with free row-sum
       (accum_out), prob emitted in bf16.
  MM2: acc_new = diag(alpha) @ acc (f32r rescale matmul) + probT @ dataset
       (bf16); probT via PE transposes (xbar DMA transpose races when
       pipelined - keep USE_XBAR=False).
  merge: AllReduce-max of m, rescale by exp(u(m_loc-m_glob)),
         AllReduce-add of [acc | l], then v = dcoef*acc/l + vcoef*x.
"""

import sys

sys.path.insert(0, "/opt/trn_rl_repo")

import numpy as np

import concourse.bass as bass
import concourse.mybir as mybir
import concourse.tile as tile
from concourse import bacc
from concourse.bass_utils import run_bass_kernel_spmd
from concourse.masks import make_identity

B, D = 1024, 512
NCORES = 8
NTILE = 512  # dataset rows per n-tile
NBT = B // 128  # 8 b-tiles

F32 = mybir.dt.float32
F32R = mybir.dt.float32r
BF16 = mybir.dt.bfloat16

AF = mybir.ActivationFunctionType
OP = mybir.AluOpType
AX = mybir.AxisListType

DEBUG = False
USE_XBAR = False
LINEARIZE = False
MM1_MODE = "bsplit"  # "fp32" | "split" | "bsplit"
SIM_1CORE = False  # build single-core, no collectives (for TimelineSim)
SIM_SKIP = set()  # sim-only op omission for time attribution
BUFS_NAT = 2
BUFS_DT = 2
BUFS_SF = 5
BUFS_DN = 2
BUFS_TINY = 4
ACC_COPY_DVE = False
BUFS_PSL = 3
BUFS_PSA = 2
BUFS_PST = 2
BUFS_SQ = 2


def build(n_tiles):
    n_sh = n_tiles * NTILE
    split = MM1_MODE in ("split", "bsplit")
    SDT = BF16 if MM1_MODE == "bsplit" else F32R  # split operand dtype
    ndev = 1 if SIM_1CORE else NCORES
    nc = bacc.Bacc("TRN2", target_bir_lowering=False, debug=False, num_devices=ndev)

    ds = nc.declare_dram_parameter("dataset", [n_sh, D], F32, isOutput=False)
    xt = nc.declare_dram_parameter("x_t", [B, D], F32, isOutput=False)
    # per-b coefficient vectors, column layout [128, 8]: col i holds b = i*128+p
    ucol_p = nc.declare_dram_parameter("ucol", [128, NBT], F32, isOutput=False)
    nucol_p = nc.declare_dram_parameter("nucol", [128, NBT], F32, isOutput=False)
    dcol_p = nc.declare_dram_parameter("dcol", [128, NBT], F32, isOutput=False)
    vcol_p = nc.declare_dram_parameter("vcol", [128, NBT], F32, isOutput=False)
    # whalf = -(a/2) as a row [1, B]
    wrow_p = nc.declare_dram_parameter("wrow", [1, B], F32, isOutput=False)
    out = nc.declare_dram_parameter("out", [B, D], F32, isOutput=True)
    if DEBUG:
        dbg_m = nc.declare_dram_parameter("dbg_m", [128, NBT], F32, isOutput=True)
        dbg_l = nc.declare_dram_parameter("dbg_l", [128, NBT], F32, isOutput=True)
        dbg_acc = nc.declare_dram_parameter("dbg_acc", [128, D], F32, isOutput=True)
        dbg_pl = nc.declare_dram_parameter("dbg_pl", [128, NTILE], F32, isOutput=True)

    ds_t = ds.ap().rearrange("(t j p) d -> t j p d", j=4, p=128)  # [nt, 4, 128, 512]
    xt_t = xt.ap().rearrange("(i p) d -> i p d", p=128)  # [8, 128, 512]
    out_t = out.ap().rearrange("(i p) d -> i p d", p=128)

    with tile.TileContext(nc, linearize=LINEARIZE) as tc:
        with (
            tc.tile_pool(name="persist", bufs=1) as pp,
            tc.tile_pool(name="xn", bufs=2) as xnp,
            tc.tile_pool(name="nat", bufs=BUFS_NAT) as natp,
            tc.tile_pool(name="natbf", bufs=BUFS_NAT) as natbfp,
            tc.tile_pool(name="dt", bufs=BUFS_DT) as dtp,
            tc.tile_pool(name="sq", bufs=BUFS_SQ) as sqp,
            tc.tile_pool(name="res", bufs=2) as resp,
            tc.tile_pool(name="sf", bufs=BUFS_SF) as sfp,
            tc.tile_pool(name="dn", bufs=BUFS_DN) as dnp,
            tc.tile_pool(name="tiny", bufs=BUFS_TINY) as tp,
            tc.tile_pool(name="fin", bufs=2) as finp,
            tc.tile_pool(name="psL", bufs=BUFS_PSL, space="PSUM") as psL,
            tc.tile_pool(name="psA", bufs=BUFS_PSA, space="PSUM") as psA,
            tc.tile_pool(name="psT", bufs=BUFS_PST, space="PSUM") as psT,
            tc.tile_pool(name="dram", bufs=1, space="DRAM") as dram,
        ):
            # ---------------- constants / setup ----------------
            ident = pp.tile([128, 128], F32)
            make_identity(nc, ident[:])
            ident_bf = pp.tile([128, 128], BF16)
            nc.vector.tensor_copy(ident_bf[:], ident[:])
            ones_f = pp.tile([128, 1], F32)
            nc.vector.memset(ones_f[:], 1.0)

            ucol = pp.tile([128, NBT], F32)
            nucol = pp.tile([128, NBT], F32)
            dcol = pp.tile([128, NBT], F32)
            vcol = pp.tile([128, NBT], F32)
            for t_, p_ in ((ucol, ucol_p), (nucol, nucol_p), (dcol, dcol_p), (vcol, vcol_p)):
                nc.sync.dma_start(out=t_[:], in_=p_.ap())

            wrow = pp.tile([1, B], F32)
            nc.sync.dma_start(out=wrow[:], in_=wrow_p.ap())
            if MM1_MODE == "split":
                # whalf hi/lo f32r rows -> w3 [3, B] = (wh, wh, wl)
                w3 = pp.tile([3, B], F32R)
                wh = pp.tile([1, B], F32R)
                wres = pp.tile([1, B], F32)
                wl = pp.tile([1, B], F32R)
                nc.vector.tensor_copy(wh[:], wrow[:])
                nc.vector.tensor_tensor(wres[:], wrow[:], wh[:], op=OP.subtract)
                nc.vector.tensor_copy(wl[:], wres[:])
                nc.sync.dma_start(out=w3[0:1, :], in_=wh[:])
                nc.sync.dma_start(out=w3[1:2, :], in_=wh[:])
                nc.sync.dma_start(out=w3[2:3, :], in_=wl[:])
            elif MM1_MODE == "bsplit":
                # 3-way bf16 split of whalf: rows (w1,w1,w1,w2,w2,w3)
                w3 = pp.tile([6, B], BF16)
                wsp = [pp.tile([1, B], BF16, name=f"wsp{j}") for j in range(3)]
                wr1 = pp.tile([1, B], F32)
                wr2 = pp.tile([1, B], F32)
                nc.vector.tensor_copy(wsp[0][:], wrow[:])
                nc.vector.tensor_tensor(wr1[:], wrow[:], wsp[0][:], op=OP.subtract)
                nc.vector.tensor_copy(wsp[1][:], wr1[:])
                nc.vector.tensor_tensor(wr2[:], wr1[:], wsp[1][:], op=OP.subtract)
                nc.vector.tensor_copy(wsp[2][:], wr2[:])
                for r, j in enumerate((0, 0, 0, 1, 1, 2)):
                    nc.sync.dma_start(out=w3[r:r + 1, :], in_=wsp[j][:])

            # x_tT (+ residual split): [4][128, B]
            xdt = SDT if split else F32
            xh = [pp.tile([128, B], xdt, tag=f"xh{k}", name=f"xh{k}") for k in range(4)]
            if split:
                xl = [pp.tile([128, B], SDT, tag=f"xl{k}", name=f"xl{k}") for k in range(4)]
            for i in range(NBT):
                xnat = xnp.tile([128, D], F32, tag="xnat")
                nc.sync.dma_start(out=xnat[:], in_=xt_t[i])
                for k in range(4):
                    pX = psT.tile([128, 128], F32, tag="pT", name="pX")
                    nc.tensor.transpose(pX[:], xnat[:, k * 128:(k + 1) * 128], ident[:])
                    sl = (slice(None), slice(i * 128, (i + 1) * 128))
                    nc.scalar.copy(xh[k][sl], pX[:])
                    if split:
                        rX = resp.tile([128, 128], F32, tag="rX")
                        nc.vector.tensor_tensor(rX[:], pX[:], xh[k][sl], op=OP.subtract)
                        nc.vector.tensor_copy(xl[k][sl], rX[:])

            # running stats
            m_run = pp.tile([128, NBT], F32)
            l_run = pp.tile([128, NBT], F32)
            acc = [pp.tile([128, D], F32R, tag=f"acc{i}", name=f"acc{i}") for i in range(NBT)]
            nc.vector.memset(m_run[:], -1.0e30)
            nc.vector.memset(l_run[:], 0.0)
            for i in range(NBT):
                nc.vector.memset(acc[i][:].bitcast(F32), 0.0)

            dn_dram = dram.tile([2, n_sh], F32R)

            # ---------------- main loop over dataset tiles ----------------
            for t in range(n_tiles):
                nat = [natp.tile([128, D], F32, tag=f"nat{j}", name=f"nat{j}") for j in range(4)]
                natbf = [natbfp.tile([128, D], BF16, tag=f"natbf{j}", name=f"natbf{j}") for j in range(4)]
                for j in range(4):
                    nc.sync.dma_start(out=nat[j][:], in_=ds_t[t, j])
                    if "natbf" not in SIM_SKIP:
                        nc.gpsimd.tensor_copy(natbf[j][:], nat[j][:])

                # transposed dataset chunks dT* [4][128d, 512n], and
                # dn row = sum_d dataT^2 via fp32 ones-matmul on Square(dataT)
                ddt = SDT if split else F32
                pD = psT.tile([1, NTILE], F32, tag="pT", name="pD")
                dTh = [dtp.tile([128, NTILE], ddt, tag=f"dTh{k}", name=f"dTh{k}") for k in range(4)]
                if split:
                    dTl = [dtp.tile([128, NTILE], SDT, tag=f"dTl{k}", name=f"dTl{k}") for k in range(4)]
                for k in range(4):
                    pT = psT.tile([128, NTILE], F32, tag="pT")
                    if "dtr" not in SIM_SKIP:
                        for j in range(4):
                            nc.tensor.transpose(
                                pT[:, j * 128:(j + 1) * 128],
                                nat[j][:, k * 128:(k + 1) * 128],
                                ident[:],
                            )
                    if "dtcast" not in SIM_SKIP:
                        nc.scalar.copy(dTh[k][:], pT[:])
                        if split:
                            rT = resp.tile([128, NTILE], F32, tag="rT")
                            nc.vector.tensor_tensor(rT[:], pT[:], dTh[k][:], op=OP.subtract)
                            nc.vector.tensor_copy(dTl[k][:], rT[:])
                    if "dn" not in SIM_SKIP:
                        sq = sqp.tile([128, D], F32, tag="sq")
                        nc.scalar.activation(sq[:], pT[:], AF.Square)
                        nc.tensor.matmul(
                            pD[:], ones_f[:], sq[:], start=(k == 0), stop=(k == 3)
                        )
                sl_n = slice(t * NTILE, (t + 1) * NTILE)
                if MM1_MODE == "bsplit":
                    dnf = dnp.tile([1, NTILE], F32, tag="dnf")
                    nc.scalar.copy(dnf[:], pD[:])
                    d1 = dnp.tile([1, NTILE], BF16, tag="d1")
                    d2 = dnp.tile([1, NTILE], BF16, tag="d2")
                    d3 = dnp.tile([1, NTILE], BF16, tag="d3")
                    r1 = dnp.tile([1, NTILE], F32, tag="r1")
                    r2 = dnp.tile([1, NTILE], F32, tag="r2")
                    nc.vector.tensor_copy(d1[:], dnf[:])
                    nc.vector.tensor_tensor(r1[:], dnf[:], d1[:], op=OP.subtract)
                    nc.vector.tensor_copy(d2[:], r1[:])
                    nc.vector.tensor_tensor(r2[:], r1[:], d2[:], op=OP.subtract)
                    nc.vector.tensor_copy(d3[:], r2[:])
                    # dnK rows = (dn1,dn2,dn3,dn1,dn2,dn1) via direct SBUF DMAs
                    dnK = dnp.tile([6, NTILE], BF16, tag="dnK")
                    for r, src in enumerate((d1, d2, d3, d1, d2, d1)):
                        nc.sync.dma_start(out=dnK[r:r + 1, :], in_=src[:])
                elif split:
                    dnh_row = dnp.tile([1, NTILE], F32R, tag="dnh_row")
                    dnr_row = dnp.tile([1, NTILE], F32, tag="dnr_row")
                    dnl_row = dnp.tile([1, NTILE], F32R, tag="dnl_row")
                    nc.scalar.copy(dnh_row[:], pD[:])
                    nc.vector.tensor_tensor(dnr_row[:], pD[:], dnh_row[:], op=OP.subtract)
                    nc.vector.tensor_copy(dnl_row[:], dnr_row[:])
                    nc.sync.dma_start(out=dn_dram[0, sl_n], in_=dnh_row[:])
                    nc.sync.dma_start(out=dn_dram[1, sl_n], in_=dnl_row[:])
                    dnK = dnp.tile([3, NTILE], F32R, tag="dnK")
                    nc.sync.dma_start(out=dnK[0:2, :], in_=dn_dram[:, sl_n])
                    nc.sync.dma_start(out=dnK[2:3, :], in_=dn_dram[0:1, sl_n])
                else:
                    dnh_row = dnp.tile([1, NTILE], F32, tag="dnh_row")
                    nc.scalar.copy(dnh_row[:], pD[:])
                    nc.sync.dma_start(out=dn_dram[0, sl_n].bitcast(F32), in_=dnh_row[:])
                    dnK = dnp.tile([1, NTILE], F32, tag="dnK")
                    nc.sync.dma_start(out=dnK[:], in_=dn_dram[0, sl_n].bitcast(F32))

                # per b-tile: MM1, online softmax, MM2
                for i in range(NBT):
                    bi = slice(i * 128, (i + 1) * 128)
                    pL = psL.tile([128, NTILE], F32, tag="pL")
                    first = True
                    passes = ((xh, dTh), (xh, dTl), (xl, dTh)) if split else ((xh, dTh),)
                    if "mm1" not in SIM_SKIP:
                        for hk, dk in passes:
                            for k in range(4):
                                nc.tensor.matmul(
                                    pL[:], hk[k][:, bi], dk[k][:],
                                    start=first, stop=False,
                                )
                                first = False
                    wK = w3 if split else wrow
                    nc.tensor.matmul(pL[:], wK[:, bi], dnK[:], start=first, stop=True)
                    if DEBUG and t == 0 and i == 0:
                        plc = finp.tile([128, NTILE], F32, tag="accs", name="plc")
                        nc.scalar.copy(plc[:], pL[:])
                        nc.sync.dma_start(out=dbg_pl.ap(), in_=plc[:])

                    # online max update
                    if "stats" in SIM_SKIP:
                        continue
                    mt = tp.tile([128, 1], F32, tag="mt")
                    nc.vector.tensor_reduce(mt[:], pL[:], axis=AX.X, op=OP.max)
                    dlt = tp.tile([128, 1], F32, tag="dlt")
                    # dlt = min(m_old - mt, 0) = m_old - m_new
                    nc.vector.tensor_scalar(
                        out=dlt[:], in0=m_run[:, i:i + 1], scalar1=mt[:],
                        scalar2=0.0, op0=OP.subtract, op1=OP.min,
                    )
                    nc.vector.tensor_tensor(
                        m_run[:, i:i + 1], m_run[:, i:i + 1], mt[:], op=OP.max
                    )
                    alpha = tp.tile([128, 1], F32, tag="alpha")
                    nc.scalar.activation(
                        alpha[:], dlt[:], AF.Exp, bias=0.0, scale=ucol[:, i:i + 1]
                    )
                    # bias = -u * m_new
                    ebias = tp.tile([128, 1], F32, tag="ebias")
                    nc.vector.tensor_tensor(
                        ebias[:], nucol[:, i:i + 1], m_run[:, i:i + 1], op=OP.mult
                    )
                    # prob = exp(u*P + bias), lt = rowsum
                    prob = sfp.tile([128, NTILE], BF16, tag="prob")
                    lt = tp.tile([128, 1], F32, tag="lt")
                    nc.scalar.activation(
                        prob[:], pL[:], AF.Exp,
                        bias=ebias[:], scale=ucol[:, i:i + 1], accum_out=lt[:],
                    )
                    # l = l*alpha + lt (fused)
                    nc.vector.scalar_tensor_tensor(
                        out=l_run[:, i:i + 1], in0=l_run[:, i:i + 1],
                        scalar=alpha[:], in1=lt[:], op0=OP.mult, op1=OP.add,
                    )
                    # probT transpose (bf16): xbar DMA or PE fallback
                    if "tail" in SIM_SKIP:
                        continue
                    probT = sfp.tile([128, NTILE], BF16, tag="probT")
                    if USE_XBAR:
                        for k in range(4):
                            ksl = slice(k * 128, (k + 1) * 128)
                            nc.sync.dma_start_transpose(probT[:, ksl], prob[:, ksl])
                    else:
                        pP = psA.tile([128, NTILE], BF16, tag="pA", name="pP")
                        for k in range(4):
                            ksl = slice(k * 128, (k + 1) * 128)
                            nc.tensor.transpose(pP[:, ksl], prob[:, ksl], ident_bf[:])
                        nc.scalar.copy(probT[:], pP[:])
                    # diag(alpha) as f32r
                    diag = sfp.tile([128, 128], F32R, tag="diag")
                    nc.vector.tensor_scalar(
                        out=diag[:], in0=ident[:], scalar1=alpha[:],
                        scalar2=None, op0=OP.mult,
                    )
                    # MM2: acc_new = diag @ acc + probT-chunks @ natbf
                    pA = psA.tile([128, D], F32, tag="pA")
                    nc.tensor.matmul(pA[:], diag[:], acc[i][:], start=True, stop=False)
                    for k in range(4):
                        ksl = slice(k * 128, (k + 1) * 128)
                        nc.tensor.matmul(
                            pA[:], probT[:, ksl], natbf[k][:],
                            start=False, stop=(k == 3),
                        )
                    if ACC_COPY_DVE:
                        nc.vector.tensor_copy(acc[i][:], pA[:])
                    else:
                        nc.scalar.copy(acc[i][:], pA[:])

            if DEBUG:
                nc.sync.dma_start(out=dbg_m.ap(), in_=m_run[:])
                nc.sync.dma_start(out=dbg_l.ap(), in_=l_run[:])
                acc0c = finp.tile([128, D], F32, tag="accs", name="acc0c")
                nc.vector.tensor_copy(acc0c[:], acc[0][:])
                nc.sync.dma_start(out=dbg_acc.ap(), in_=acc0c[:])

            # ---------------- cross-core merge ----------------
            m_cc_in = dram.tile([128, NBT], F32)
            m_cc_out = dram.tile([128, NBT], F32)
            nc.sync.dma_start(out=m_cc_in[:], in_=m_run[:])
            if not SIM_1CORE:
                nc.gpsimd.collective_compute(
                    "AllReduce", OP.max,
                    replica_groups=[list(range(NCORES))],
                    ins=[m_cc_in[:].opt()], outs=[m_cc_out[:].opt()],
                )
            else:
                nc.sync.dma_start(out=m_cc_out[:], in_=m_cc_in[:])
            m_glob = pp.tile([128, NBT], F32)
            nc.sync.dma_start(out=m_glob[:], in_=m_cc_out[:])

            # gamma_i = exp(u * (m_loc - m_glob)); scale acc, l
            dg = pp.tile([128, NBT], F32)
            nc.vector.tensor_tensor(dg[:], m_run[:], m_glob[:], op=OP.subtract)
            gam = pp.tile([128, NBT], F32)
            for i in range(NBT):
                nc.scalar.activation(
                    gam[:, i:i + 1], dg[:, i:i + 1], AF.Exp,
                    bias=0.0, scale=ucol[:, i:i + 1],
                )
            nc.vector.tensor_tensor(l_run[:], l_run[:], gam[:], op=OP.mult)

            accl_in = dram.tile([128, NBT * D + NBT], F32)
            accl_out = dram.tile([128, NBT * D + NBT], F32)
            for i in range(NBT):
                accs = finp.tile([128, D], F32, tag="accs")
                nc.vector.tensor_scalar(
                    out=accs[:], in0=acc[i][:], scalar1=gam[:, i:i + 1],
                    scalar2=None, op0=OP.mult,
                )
                nc.sync.dma_start(out=accl_in[:, i * D:(i + 1) * D], in_=accs[:])
            nc.sync.dma_start(out=accl_in[:, NBT * D:], in_=l_run[:])
            if not SIM_1CORE:
                nc.gpsimd.collective_compute(
                    "AllReduce", OP.add,
                    replica_groups=[list(range(NCORES))],
                    ins=[accl_in[:].opt()], outs=[accl_out[:].opt()],
                )
            else:
                nc.sync.dma_start(out=accl_out[:], in_=accl_in[:])

            lg = pp.tile([128, NBT], F32)
            nc.sync.dma_start(out=lg[:], in_=accl_out[:, NBT * D:])
            rl = pp.tile([128, NBT], F32)
            nc.vector.reciprocal(rl[:], lg[:])
            # s1 = dcoef / l
            s1 = pp.tile([128, NBT], F32)
            nc.vector.tensor_tensor(s1[:], dcol[:], rl[:], op=OP.mult)
            for i in range(NBT):
                accg = finp.tile([128, D], F32, tag="accg")
                nc.sync.dma_start(out=accg[:], in_=accl_out[:, i * D:(i + 1) * D])
                xnat = xnp.tile([128, D], F32, tag="xnat")
                nc.sync.dma_start(out=xnat[:], in_=xt_t[i])
                v1 = finp.tile([128, D], F32, tag="v1")
                nc.vector.tensor_scalar(
                    out=v1[:], in0=accg[:], scalar1=s1[:, i:i + 1],
                    scalar2=None, op0=OP.mult,
                )
                v2 = finp.tile([128, D], F32, tag="v2")
                nc.vector.tensor_scalar(
                    out=v2[:], in0=xnat[:], scalar1=vcol[:, i:i + 1],
                    scalar2=None, op0=OP.mult,
                )
                nc.vector.tensor_tensor(v1[:], v1[:], v2[:], op=OP.add)
                nc.sync.dma_start(out=out_t[i], in_=v1[:])

    nc.compile()
    return nc


_BUILD_CACHE = {}


def _get_nc(n_tiles):
    key = (n_tiles, MM1_MODE, USE_XBAR, LINEARIZE, DEBUG, SIM_1CORE, BUFS_NAT, BUFS_DT, BUFS_SF, BUFS_DN, BUFS_TINY, ACC_COPY_DVE, BUFS_PSL, BUFS_PSA, BUFS_PST, BUFS_SQ)
    if key not in _BUILD_CACHE:
        _BUILD_CACHE[key] = build(n_tiles)
    return _BUILD_CACHE[key]


def make_in_maps(x_t, t, dataset, n_tiles):
    """Shard + pad dataset, compute coefficient vectors."""
    n = dataset.shape[0]
    n_pad = NCORES * n_tiles * NTILE
    assert n_pad >= n
    dpad = np.zeros((n_pad, D), dtype=np.float32)
    dpad[:n] = dataset
    dpad[n:, 0] = 1000.0  # far-away pad rows: huge norm, ~zero softmax weight
    shards = dpad.reshape(NCORES, n_tiles * NTILE, D)

    a = t.astype(np.float64)
    b = 1.0 - a
    u = (a / (b * b)).astype(np.float32)
    w = (-a / 2.0).astype(np.float32)
    dcoef = (1.0 + a / b).astype(np.float32)
    vcoef = (-1.0 / b).astype(np.float32)

    def col(v):
        return np.ascontiguousarray(v.reshape(NBT, 128).T)

    base = dict(
        x_t=np.ascontiguousarray(x_t),
        ucol=col(u),
        nucol=col(-u),
        dcol=col(dcoef),
        vcol=col(vcoef),
        wrow=np.ascontiguousarray(w.reshape(1, B)),
    )
    return [dict(base, dataset=np.ascontiguousarray(shards[c])) for c in range(NCORES)]


def kernel(x_t, t, dataset):
    x_t = np.asarray(x_t, dtype=np.float32)
    t = np.asarray(t, dtype=np.float32)
    dataset = np.asarray(dataset, dtype=np.float32)
    n = dataset.shape[0]
    n_tiles = -(-n // (NCORES * NTILE))  # ceil -> 25 for N=100000
    nc = _get_nc(n_tiles)
    in_maps = make_in_maps(x_t, t, dataset, n_tiles)
    res = run_bass_kernel_spmd(nc, in_maps, core_ids=list(range(NCORES)))
    return np.asarray(res.results[0]["out"], dtype=np.float32)


def ref_numpy(x_t, t, dataset):
    aa = t.astype(np.float64)
    bb = 1.0 - aa
    dsn = (dataset.astype(np.float64) ** 2).sum(1)
    t2 = x_t.astype(np.float64) @ dataset.T.astype(np.float64)
    logit = (-1.0 / (2 * bb * bb))[:, None] * (
        (x_t.astype(np.float64) ** 2).sum(1)[:, None]
        - 2 * aa[:, None] * t2
        + (aa * aa)[:, None] * dsn[None, :]
    )
    p = np.exp(logit - logit.max(1, keepdims=True))
    p /= p.sum(1, keepdims=True)
    p = p * (1 + aa / bb)[:, None]
    return (-1.0 / bb)[:, None] * x_t.astype(np.float64) + p @ dataset.astype(np.float64)


if __name__ == "__main__":
    rng = np.random.default_rng(0)
    n = 2 * NCORES * NTILE - 300
    x_t = rng.standard_normal((B, D)).astype(np.float32)
    t = rng.uniform(0.05, 0.95, (B,)).astype(np.float32)
    dataset = rng.standard_normal((n, D)).astype(np.float32)
    v = kernel(x_t, t, dataset)
    vref = ref_numpy(x_t, t, dataset)
    err = np.linalg.norm(v - vref) / np.linalg.norm(vref)
    print("rel l2 err:", err)
    print("max abs err:", np.abs(v - vref).max(), "ref scale:", np.abs(vref).max())

